# revision 1
# baseline (speedup 1.0000x reference)
"""Trainium2 Bass kernel for NRI-style GNN decoder (nn_Decoder_58600533787128).

Data-parallel over batch across 8 NeuronCores.  All awkward layout work
(transposes, edge padding, bias broadcast) happens host-side in numpy; the
device program is pure matmul/activation steady-state.

Per-core math (bpc=8 batches per core), per batch b:
  pre^T[c,e]   = gather: senders^T = x^T-gather via matmul(lhsT=x, rhs=rel_send^T)
  h1^T[h,e]    = relu(W1^T-chunks @ pre^T + b1)        (4 edge types)
  m[e,o]       = h1^T-chunks as lhsT @ W2-chunks        (accum over h)
  msc[e,o]     = relu(m + b2) * rel_type[b,e,i]         (scale>=0 folds into relu)
  agg^T[o,n]  += msc^T... via matmul(lhsT=msc, rhs=rel_rec-tile)   (accum 32x4)
  y^T[o,n]     = MLP(aug^T = [x^T; agg^T])              (output MLP)
Host transposes y^T back to [n,o].
"""
import sys

sys.path.insert(0, "/opt/trn_rl_repo")

import numpy as np

B, N, F, H, O, T, E = 64, 64, 64, 256, 64, 4, 4032
EP = 4096          # padded edge count
NT = EP // 128     # 32 edge tiles of 128
NCORES = 8
BPC = B // NCORES  # batches per core


def build_nc(bpc=BPC, num_devices=NCORES, reps=1):
    import concourse.mybir as mybir
    from concourse import bacc, tile

    dt = mybir.dt.float32
    AF = mybir.ActivationFunctionType
    ALU = mybir.AluOpType

    nc = bacc.Bacc(
        "TRN2", target_bir_lowering=False, debug=False, num_devices=num_devices
    )
    x_d = nc.declare_dram_parameter("x", [bpc, N, F], dt, isOutput=False)
    xT_d = nc.declare_dram_parameter("xT", [bpc, F, N], dt, isOutput=False)
    rt_d = nc.declare_dram_parameter("rt", [bpc, 128, NT * T], dt, isOutput=False)
    rsT_d = nc.declare_dram_parameter("rsT", [N, EP], dt, isOutput=False)
    rrT_d = nc.declare_dram_parameter("rrT", [N, EP], dt, isOutput=False)
    rrec_d = nc.declare_dram_parameter("rrec", [128, NT * N], dt, isOutput=False)
    w1_d = nc.declare_dram_parameter("w1s", [128, T * H], dt, isOutput=False)
    w2_d = nc.declare_dram_parameter("w2s", [128, T * 2 * O], dt, isOutput=False)
    b1_d = nc.declare_dram_parameter("b1c", [128, T * 2], dt, isOutput=False)
    b2_d = nc.declare_dram_parameter("b2r", [128, T * 4 * O], dt, isOutput=False)
    ow1_d = nc.declare_dram_parameter("ow1s", [128, H], dt, isOutput=False)
    ob1_d = nc.declare_dram_parameter("ob1c", [128, 2], dt, isOutput=False)
    ow2_d = nc.declare_dram_parameter("ow2s", [128, 2 * H], dt, isOutput=False)
    ob2_d = nc.declare_dram_parameter("ob2c", [128, 2], dt, isOutput=False)
    ow3_d = nc.declare_dram_parameter("ow3s", [128, 2 * O], dt, isOutput=False)
    ob3_d = nc.declare_dram_parameter("ob3c", [O, 1], dt, isOutput=False)
    y_d = nc.declare_dram_parameter("y", [bpc, O, N], dt, isOutput=True)

    with tile.TileContext(nc) as tc:
        with (
            tc.tile_pool(name="const", bufs=1) as cpool,
            tc.tile_pool(name="work", bufs=4) as wpool,
            tc.tile_pool(name="h1pool", bufs=4) as hpool,
            tc.tile_pool(name="ppre", bufs=2, space="PSUM") as ppre,
            tc.tile_pool(name="ph1", bufs=2, space="PSUM") as ph1,
            tc.tile_pool(name="pm", bufs=2, space="PSUM") as pm,
            tc.tile_pool(name="pagg", bufs=2, space="PSUM") as pagg,
        ):
            # resident constants (one DMA each; layouts prepped host-side)
            rsT = cpool.tile([N, EP], dt)
            nc.sync.dma_start(rsT[:], rsT_d[:])
            rrT = cpool.tile([N, EP], dt)
            nc.sync.dma_start(rrT[:], rrT_d[:])
            rrec = cpool.tile([128, NT * N], dt)
            nc.sync.dma_start(rrec[:], rrec_d[:])
            w1s = cpool.tile([128, T * H], dt)
            nc.sync.dma_start(w1s[:], w1_d[:])
            w2s = cpool.tile([128, T * 2 * O], dt)
            nc.sync.dma_start(w2s[:], w2_d[:])
            b1c = cpool.tile([128, T * 2], dt)
            nc.sync.dma_start(b1c[:], b1_d[:])
            b2r = cpool.tile([128, T * 4 * O], dt)
            nc.sync.dma_start(b2r[:], b2_d[:])
            ow1s = cpool.tile([128, H], dt)
            nc.sync.dma_start(ow1s[:], ow1_d[:])
            ob1c = cpool.tile([128, 2], dt)
            nc.sync.dma_start(ob1c[:], ob1_d[:])
            ow2s = cpool.tile([128, 2 * H], dt)
            nc.sync.dma_start(ow2s[:], ow2_d[:])
            ob2c = cpool.tile([128, 2], dt)
            nc.sync.dma_start(ob2c[:], ob2_d[:])
            ow3s = cpool.tile([128, 2 * O], dt)
            nc.sync.dma_start(ow3s[:], ow3_d[:])
            ob3c = cpool.tile([O, 1], dt)
            nc.sync.dma_start(ob3c[:], ob3_d[:])
            ones_sb = cpool.tile([1, 128], dt)
            nc.gpsimd.memset(ones_sb[:], 1.0)

            import contextlib
            loop_cm = tc.For_i(0, reps, 1) if reps > 1 else contextlib.nullcontext()
            with loop_cm:
              for b in range(bpc):
                x_sb = wpool.tile([N, F], dt, tag="x_sb")
                nc.sync.dma_start(x_sb[:], x_d[b])
                augT = wpool.tile([128, N], dt, tag="augT")
                nc.sync.dma_start(augT[0:F, :], xT_d[b])
                rt_sb = wpool.tile([128, NT * T], dt, tag="rt_sb")
                nc.sync.dma_start(rt_sb[:], rt_d[b])

                aggp = pagg.tile([O, N], dt, tag="aggp")
                NST = EP // 512  # 8 supertiles of 512 edges (4 psum-subtiles each)
                for st in range(NST):
                    e0 = st * 512
                    # gather: pre^T = [senders^T ; receivers^T], N=512 (2 col-groups)
                    prep = ppre.tile([128, 512], dt, tag="prep")
                    nc.tensor.matmul(
                        prep[0:64, :], x_sb[:], rsT[:, e0:e0 + 512],
                        start=True, stop=True,
                    )
                    nc.tensor.matmul(
                        prep[64:128, :], x_sb[:], rrT[:, e0:e0 + 512],
                        start=True, stop=True,
                    )
                    preT = wpool.tile([128, 512], dt, tag="preT")
                    nc.vector.tensor_copy(preT[:], prep[:])

                    for i in range(T):
                        h1s = hpool.tile([128, 2 * 512], dt, tag="h1s")
                        for hc in range(2):
                            h1p = ph1.tile([128, 512], dt, tag="h1p")
                            nc.tensor.matmul(
                                h1p[:],
                                w1s[:, i * H + hc * 128: i * H + (hc + 1) * 128],
                                preT[:], start=True, stop=True,
                            )
                            g = i * 2 + hc
                            dst = h1s[:, hc * 512:(hc + 1) * 512]
                            bias = b1c[:, g:g + 1]
                            if g % 4 == 1:  # balance: 2 of 8 chunks on DVE
                                nc.vector.tensor_scalar(
                                    dst, h1p[:], bias, 0.0, ALU.add, ALU.max
                                )
                            else:
                                nc.scalar.activation(dst, h1p[:], AF.Relu, bias=bias)
                        # layer2: 4 psum-subtiles of 128 edges in one bank
                        mp = pm.tile([128, 4 * O], dt, tag="mp")
                        # init with b2 broadcast over all 128 edge-partitions (K=1 ones)
                        nc.tensor.matmul(
                            mp[:], ones_sb[:], b2r[0:1, i * 4 * O:(i + 1) * 4 * O],
                            start=True, stop=False, skip_group_check=True,
                        )
                        for sub in range(4):
                            for kc in range(2):
                                nc.tensor.matmul(
                                    mp[:, sub * O:(sub + 1) * O],
                                    h1s[:, kc * 512 + sub * 128: kc * 512 + (sub + 1) * 128],
                                    w2s[:, (i * 2 + kc) * O:(i * 2 + kc + 1) * O],
                                    start=False, stop=(sub == 3 and kc == 1),
                                    skip_group_check=True,
                                )
                        msc = hpool.tile([128, 4 * O], dt, tag="msc")
                        for sub in range(4):
                            et = st * 4 + sub
                            # relu(m + b2) * rt  ==  (mp max 0) * rt  (rt >= 0)
                            nc.vector.tensor_scalar(
                                msc[:, sub * O:(sub + 1) * O],
                                mp[:, sub * O:(sub + 1) * O],
                                0.0, rt_sb[:, et * T + i: et * T + i + 1],
                                ALU.max, ALU.mult,
                            )
                        for sub in range(4):
                            et = st * 4 + sub
                            nc.tensor.matmul(
                                aggp[:], msc[:, sub * O:(sub + 1) * O],
                                rrec[:, et * N:(et + 1) * N],
                                start=(et == 0 and i == 0),
                                stop=(et == NT - 1 and i == T - 1),
                                skip_group_check=True,
                            )

                # output MLP on aug^T = [x^T ; agg^T]
                nc.vector.tensor_copy(augT[F:128, :], aggp[:])
                f1 = wpool.tile([128, 2 * N], dt, tag="f1")
                for mc in range(2):
                    fp = pm.tile([128, O], dt, tag="mp")
                    nc.tensor.matmul(
                        fp[:, 0:N], ow1s[:, mc * 128:(mc + 1) * 128], augT[:],
                        start=True, stop=True,
                    )
                    nc.scalar.activation(
                        f1[:, mc * N:(mc + 1) * N], fp[:, 0:N], AF.Relu,
                        bias=ob1c[:, mc:mc + 1],
                    )
                f2 = wpool.tile([128, 2 * N], dt, tag="f2")
                for mc in range(2):
                    fp = pm.tile([128, O], dt, tag="mp")
                    for kc in range(2):
                        nc.tensor.matmul(
                            fp[:, 0:N],
                            ow2s[:, kc * H + mc * 128: kc * H + (mc + 1) * 128],
                            f1[:, kc * N:(kc + 1) * N],
                            start=(kc == 0), stop=(kc == 1),
                        )
                    nc.scalar.activation(
                        f2[:, mc * N:(mc + 1) * N], fp[:, 0:N], AF.Relu,
                        bias=ob2c[:, mc:mc + 1],
                    )
                op = pm.tile([128, O], dt, tag="mp")
                for kc in range(2):
                    nc.tensor.matmul(
                        op[0:O, 0:N], ow3s[:, kc * O:(kc + 1) * O],
                        f2[:, kc * N:(kc + 1) * N],
                        start=(kc == 0), stop=(kc == 1),
                    )
                yb = wpool.tile([O, N], dt, tag="yb")
                nc.vector.tensor_scalar(
                    yb[:], op[0:O, 0:N], ob3c[:, 0:1], None, ALU.add
                )
                nc.sync.dma_start(y_d[b], yb[:])

    nc.compile()
    return nc


def prep_shared(rel_rec, rel_send, w1, b1, w2, b2, ow1, ob1, ow2, ob2, ow3, ob3):
    """Host-side layout prep for the replicated tensors."""
    f32 = np.float32
    rsT = np.zeros((N, EP), f32)
    rsT[:, :E] = np.ascontiguousarray(rel_send.T)
    rrT = np.zeros((N, EP), f32)
    rrT[:, :E] = np.ascontiguousarray(rel_rec.T)
    rrec_pad = np.zeros((EP, N), f32)
    rrec_pad[:E] = rel_rec
    # [EP, N] -> tiles [128, NT*N] : cols et*N.. hold edge-tile et
    rrec_t = np.ascontiguousarray(
        rrec_pad.reshape(NT, 128, N).transpose(1, 0, 2).reshape(128, NT * N)
    )
    w1s = np.ascontiguousarray(w1.transpose(1, 0, 2).reshape(2 * F, T * H))
    w2s = np.ascontiguousarray(
        w2.reshape(T, 2, 128, O).transpose(2, 0, 1, 3).reshape(128, T * 2 * O)
    )
    b1c = np.ascontiguousarray(b1.reshape(T, 2, 128).transpose(2, 0, 1).reshape(128, T * 2))
    b2r = np.ascontiguousarray(np.broadcast_to(
        np.tile(b2, (1, 4)).reshape(1, T * 4 * O), (128, T * 4 * O)))
    ow1s = np.ascontiguousarray(ow1)                       # [128, H]
    ob1c = np.ascontiguousarray(ob1.reshape(2, 128).T)     # [128, 2]
    ow2s = np.ascontiguousarray(ow2.reshape(2, 128, H).transpose(1, 0, 2).reshape(128, 2 * H))
    ob2c = np.ascontiguousarray(ob2.reshape(2, 128).T)
    ow3s = np.ascontiguousarray(ow3.reshape(2, 128, O).transpose(1, 0, 2).reshape(128, 2 * O))
    ob3c = np.ascontiguousarray(ob3.reshape(O, 1))
    return dict(
        rsT=rsT, rrT=rrT, rrec=rrec_t, w1s=w1s, w2s=w2s, b1c=b1c, b2r=b2r,
        ow1s=ow1s, ob1c=ob1c, ow2s=ow2s, ob2c=ob2c, ow3s=ow3s, ob3c=ob3c,
    )


def prep_batch(x, rel_type):
    """Per-core batched tensors: x, xT, rt tiles."""
    f32 = np.float32
    bpc = x.shape[0]
    xT = np.ascontiguousarray(x.transpose(0, 2, 1))
    rt_pad = np.zeros((bpc, EP, T), f32)
    rt_pad[:, :E] = rel_type
    # [bpc, EP, T] -> [bpc, 128, NT*T] : col et*T+i is rel_type for tile et, type i
    rt_t = np.ascontiguousarray(
        rt_pad.reshape(bpc, NT, 128, T).transpose(0, 2, 1, 3).reshape(bpc, 128, NT * T)
    )
    return dict(x=np.ascontiguousarray(x), xT=xT, rt=rt_t)


def kernel(**inputs):
    from concourse.bass_utils import run_bass_kernel_spmd

    f32arrs = {k: np.asarray(v, dtype=np.float32) for k, v in inputs.items()}
    shared = prep_shared(
        f32arrs["rel_rec"], f32arrs["rel_send"],
        f32arrs["w1"], f32arrs["b1"], f32arrs["w2"], f32arrs["b2"],
        f32arrs["ow1"], f32arrs["ob1"], f32arrs["ow2"], f32arrs["ob2"],
        f32arrs["ow3"], f32arrs["ob3"],
    )
    in_maps = []
    for c in range(NCORES):
        sl = slice(c * BPC, (c + 1) * BPC)
        m = dict(shared)
        m.update(prep_batch(f32arrs["x"][sl], f32arrs["rel_type"][sl]))
        in_maps.append(m)

    nc = build_nc(BPC)
    res = run_bass_kernel_spmd(nc, in_maps, list(range(NCORES)))
    # y per core: [BPC, O, N] (transposed) -> full [B, N, O]
    y = np.concatenate([res.results[c]["y"] for c in range(NCORES)], axis=0)
    return np.ascontiguousarray(y.transpose(0, 2, 1)).astype(np.float32)


if __name__ == "__main__":
    # smoke: random inputs, shape check only
    rng = np.random.default_rng(0)
    inputs = {
        "x": rng.standard_normal((B, N, F), dtype=np.float32),
        "rel_type": rng.random((B, E, T), dtype=np.float32),
        "rel_rec": np.zeros((E, N), np.float32),
        "rel_send": np.zeros((E, N), np.float32),
        "w1": rng.standard_normal((T, 2 * F, H), dtype=np.float32) * 0.1,
        "b1": rng.standard_normal((T, H), dtype=np.float32) * 0.1,
        "w2": rng.standard_normal((T, H, O), dtype=np.float32) * 0.1,
        "b2": rng.standard_normal((T, O), dtype=np.float32) * 0.1,
        "ow1": rng.standard_normal((F + O, H), dtype=np.float32) * 0.1,
        "ob1": rng.standard_normal((H,), dtype=np.float32) * 0.1,
        "ow2": rng.standard_normal((H, H), dtype=np.float32) * 0.1,
        "ob2": rng.standard_normal((H,), dtype=np.float32) * 0.1,
        "ow3": rng.standard_normal((H, O), dtype=np.float32) * 0.1,
        "ob3": rng.standard_normal((O,), dtype=np.float32) * 0.1,
    }
    y = kernel(**inputs)
    print("y", y.shape, y.dtype)



# revision 2
# speedup vs baseline: 1.0048x; 1.0048x over previous
"""Trainium2 Bass kernel for NRI-style GNN decoder (nn_Decoder_58600533787128).

Data-parallel over batch across 8 NeuronCores.  All gather/scatter structure
is folded into host-side layout prep so the device program is pure
matmul/activation steady-state in bf16 (fp32 PSUM accumulation):

  host:  preT[b]  = [senders^T ; receivers^T] on the dense (i,j) pair grid
                    (4096 pairs incl. diagonal), bf16  [128, 4096]
         rrecrt[b] = rel_type-scaled one-hot receiver scatter matrix,
                    per type, tiled by 128-pair tiles, bf16 [128, T*32*64]
                    (diagonal pairs get 0, killing self-messages)
  device per batch:
         h1^T  = relu(W1^T @ preT + b1)          4 edge types, bf16 matmul
         m     = h1-chunks^T @ W2-chunks + b2    (b2 via ones-matmul preload)
         msc   = relu(m)                          (rt scale folded into rrecrt)
         agg^T+= msc^T @ rrecrt-tiles             accum 32x4 in PSUM
         y^T   = MLP([x^T ; agg^T])               output MLP
  host:  y^T -> [b, n, o]
"""
import sys

sys.path.insert(0, "/opt/trn_rl_repo")

import numpy as np
import ml_dtypes

BF16 = ml_dtypes.bfloat16

B, N, F, H, O, T, E = 64, 64, 64, 256, 64, 4, 4032
EP = N * N         # dense pair grid (i,j), 4096, includes diagonal
NT = EP // 128     # 32 pair tiles of 128
NCORES = 8
BPC = B // NCORES  # batches per core


def build_nc(bpc=BPC, num_devices=NCORES, reps=1):
    import concourse.mybir as mybir
    from concourse import bacc, tile

    f32 = mybir.dt.float32
    bf16 = mybir.dt.bfloat16
    AF = mybir.ActivationFunctionType
    ALU = mybir.AluOpType

    nc = bacc.Bacc(
        "TRN2", target_bir_lowering=False, debug=False, num_devices=num_devices
    )
    preT_d = nc.declare_dram_parameter("preT", [bpc, 128, EP], bf16, isOutput=False)
    rrt_d = nc.declare_dram_parameter("rrt", [bpc, 128, T * NT * N], bf16, isOutput=False)
    xT_d = nc.declare_dram_parameter("xT", [bpc, F, N], bf16, isOutput=False)
    w1_d = nc.declare_dram_parameter("w1s", [128, T * H], bf16, isOutput=False)
    b1_d = nc.declare_dram_parameter("b1c", [128, T * 2], f32, isOutput=False)
    w2_d = nc.declare_dram_parameter("w2s", [128, T * 2 * O], bf16, isOutput=False)
    b2_d = nc.declare_dram_parameter("b2r", [1, T * 4 * O], bf16, isOutput=False)
    ow1_d = nc.declare_dram_parameter("ow1s", [128, H], bf16, isOutput=False)
    ob1_d = nc.declare_dram_parameter("ob1c", [128, 2], f32, isOutput=False)
    ow2_d = nc.declare_dram_parameter("ow2s", [128, 2 * H], bf16, isOutput=False)
    ob2_d = nc.declare_dram_parameter("ob2c", [128, 2], f32, isOutput=False)
    ow3_d = nc.declare_dram_parameter("ow3s", [128, 2 * O], bf16, isOutput=False)
    ob3_d = nc.declare_dram_parameter("ob3c", [O, 1], f32, isOutput=False)
    y_d = nc.declare_dram_parameter("y", [bpc, O, N], f32, isOutput=True)

    with tile.TileContext(nc) as tc:
        with (
            tc.tile_pool(name="const", bufs=1) as cpool,
            tc.tile_pool(name="data", bufs=2) as dpool,
            tc.tile_pool(name="work", bufs=3) as wpool,
            tc.tile_pool(name="h1pool", bufs=4) as hpool,
            tc.tile_pool(name="ph1", bufs=3, space="PSUM") as ph1,
            tc.tile_pool(name="pm", bufs=2, space="PSUM") as pm,
            tc.tile_pool(name="pagg", bufs=1, space="PSUM") as pagg,
        ):
            # resident constants (one DMA each; layouts prepped host-side)
            w1s = cpool.tile([128, T * H], bf16)
            nc.sync.dma_start(w1s[:], w1_d[:])
            b1c = cpool.tile([128, T * 2], f32)
            nc.sync.dma_start(b1c[:], b1_d[:])
            w2s = cpool.tile([128, T * 2 * O], bf16)
            nc.sync.dma_start(w2s[:], w2_d[:])
            b2r = cpool.tile([1, T * 4 * O], bf16)
            nc.sync.dma_start(b2r[:], b2_d[:])
            ow1s = cpool.tile([128, H], bf16)
            nc.sync.dma_start(ow1s[:], ow1_d[:])
            ob1c = cpool.tile([128, 2], f32)
            nc.sync.dma_start(ob1c[:], ob1_d[:])
            ow2s = cpool.tile([128, 2 * H], bf16)
            nc.sync.dma_start(ow2s[:], ow2_d[:])
            ob2c = cpool.tile([128, 2], f32)
            nc.sync.dma_start(ob2c[:], ob2_d[:])
            ow3s = cpool.tile([128, 2 * O], bf16)
            nc.sync.dma_start(ow3s[:], ow3_d[:])
            ob3c = cpool.tile([O, 1], f32)
            nc.sync.dma_start(ob3c[:], ob3_d[:])
            ones_sb = cpool.tile([1, 128], bf16)
            nc.gpsimd.memset(ones_sb[:], 1.0)

            # relu work is rotated across the 3 element-wise engines
            def relu_bias(eidx, dst, src, bias_col):
                e = eidx % 3
                if e == 0:
                    nc.scalar.activation(dst, src, AF.Relu, bias=bias_col)
                elif e == 1:
                    nc.vector.tensor_scalar(dst, src, bias_col, 0.0, ALU.add, ALU.max)
                else:
                    nc.gpsimd.tensor_scalar(dst, src, bias_col, 0.0, ALU.add, ALU.max)

            def relu_plain(eidx, dst, src):
                e = eidx % 3
                if e == 0:
                    nc.gpsimd.tensor_scalar(dst, src, 0.0, None, ALU.max)
                elif e == 1:
                    nc.vector.tensor_scalar(dst, src, 0.0, None, ALU.max)
                else:
                    nc.scalar.activation(dst, src, AF.Relu)

            import contextlib
            loop_cm = tc.For_i(0, reps, 1) if reps > 1 else contextlib.nullcontext()
            with loop_cm:
              for b in range(bpc):
                preT = dpool.tile([128, EP], bf16, tag="preT")
                nc.sync.dma_start(preT[:, 0:EP // 2], preT_d[b, :, 0:EP // 2])
                nc.sync.dma_start(preT[:, EP // 2:EP], preT_d[b, :, EP // 2:EP])
                rrt = dpool.tile([128, T * NT * N], bf16, tag="rrt")
                for t in range(T):
                    nc.sync.dma_start(
                        rrt[:, t * NT * N:(t + 1) * NT * N],
                        rrt_d[b, :, t * NT * N:(t + 1) * NT * N],
                    )
                augT = wpool.tile([128, N], bf16, tag="augT")
                nc.sync.dma_start(augT[0:F, :], xT_d[b])

                aggp = pagg.tile([O, N], f32, tag="aggp")
                eidx = 0
                for st in range(8):
                    e0 = st * 512
                    for t in range(T):
                        h1s = hpool.tile([128, 2 * 512], bf16, tag="h1s")
                        for hc in range(2):
                            h1p = ph1.tile([128, 512], f32, tag="h1p")
                            nc.tensor.matmul(
                                h1p[:],
                                w1s[:, t * H + hc * 128: t * H + (hc + 1) * 128],
                                preT[:, e0:e0 + 512],
                                start=True, stop=True,
                            )
                            relu_bias(
                                eidx, h1s[:, hc * 512:(hc + 1) * 512], h1p[:],
                                b1c[:, t * 2 + hc: t * 2 + hc + 1],
                            )
                            eidx += 1
                        # layer2: 4 psum-subtiles of 128 pairs in one bank
                        mp = pm.tile([128, 4 * O], f32, tag="mp")
                        # preload b2 broadcast over all 128 pair-partitions
                        nc.tensor.matmul(
                            mp[:], ones_sb[:], b2r[0:1, t * 4 * O:(t + 1) * 4 * O],
                            start=True, stop=False, skip_group_check=True,
                        )
                        for sub in range(4):
                            for kc in range(2):
                                nc.tensor.matmul(
                                    mp[:, sub * O:(sub + 1) * O],
                                    h1s[:, kc * 512 + sub * 128: kc * 512 + (sub + 1) * 128],
                                    w2s[:, (t * 2 + kc) * O:(t * 2 + kc + 1) * O],
                                    start=False, stop=(sub == 3 and kc == 1),
                                    skip_group_check=True,
                                )
                        msc = hpool.tile([128, 4 * O], bf16, tag="msc")
                        relu_plain(eidx, msc[:], mp[:])
                        eidx += 1
                        for sub in range(4):
                            et = st * 4 + sub
                            nc.tensor.matmul(
                                aggp[:], msc[:, sub * O:(sub + 1) * O],
                                rrt[:, (t * NT + et) * N:(t * NT + et + 1) * N],
                                start=(st == 0 and t == 0 and sub == 0),
                                stop=(st == 7 and t == T - 1 and sub == 3),
                                skip_group_check=True,
                            )

                # output MLP on aug^T = [x^T ; agg^T]
                nc.vector.tensor_copy(augT[F:128, :], aggp[:])
                f1 = wpool.tile([128, 2 * N], bf16, tag="f1")
                for mc in range(2):
                    fp = pm.tile([128, 4 * O], f32, tag="mp")
                    nc.tensor.matmul(
                        fp[:, 0:N], ow1s[:, mc * 128:(mc + 1) * 128], augT[:],
                        start=True, stop=True,
                    )
                    nc.scalar.activation(
                        f1[:, mc * N:(mc + 1) * N], fp[:, 0:N], AF.Relu,
                        bias=ob1c[:, mc:mc + 1],
                    )
                f2 = wpool.tile([128, 2 * N], bf16, tag="f2")
                for mc in range(2):
                    fp = pm.tile([128, 4 * O], f32, tag="mp")
                    for kc in range(2):
                        nc.tensor.matmul(
                            fp[:, 0:N],
                            ow2s[:, kc * H + mc * 128: kc * H + (mc + 1) * 128],
                            f1[:, kc * N:(kc + 1) * N],
                            start=(kc == 0), stop=(kc == 1),
                        )
                    nc.scalar.activation(
                        f2[:, mc * N:(mc + 1) * N], fp[:, 0:N], AF.Relu,
                        bias=ob2c[:, mc:mc + 1],
                    )
                op = pm.tile([128, 4 * O], f32, tag="mp")
                for kc in range(2):
                    nc.tensor.matmul(
                        op[0:O, 0:N], ow3s[:, kc * O:(kc + 1) * O],
                        f2[:, kc * N:(kc + 1) * N],
                        start=(kc == 0), stop=(kc == 1),
                    )
                yb = wpool.tile([O, N], f32, tag="yb")
                nc.vector.tensor_scalar(
                    yb[:], op[0:O, 0:N], ob3c[:, 0:1], None, ALU.add
                )
                nc.sync.dma_start(y_d[b], yb[:])

    nc.compile()
    return nc


def edge_maps(rel_rec, rel_send):
    """Pair-grid index and receiver node for each of the E directed edges."""
    send_idx = np.argmax(rel_send, axis=1).astype(np.int64)  # [E]
    rec_idx = np.argmax(rel_rec, axis=1).astype(np.int64)    # [E]
    return send_idx * N + rec_idx, rec_idx


def prep_shared(w1, b1, w2, b2, ow1, ob1, ow2, ob2, ow3, ob3):
    """Host-side layout prep for the replicated weights (bf16)."""
    w1s = np.ascontiguousarray(
        w1.transpose(1, 0, 2).reshape(2 * F, T * H)).astype(BF16)
    b1c = np.ascontiguousarray(
        b1.reshape(T, 2, 128).transpose(2, 0, 1).reshape(128, T * 2)
    ).astype(np.float32)
    w2s = np.ascontiguousarray(
        w2.reshape(T, 2, 128, O).transpose(2, 0, 1, 3).reshape(128, T * 2 * O)
    ).astype(BF16)
    b2r = np.ascontiguousarray(
        np.tile(b2, (1, 4)).reshape(1, T * 4 * O)).astype(BF16)
    ow1s = np.ascontiguousarray(ow1).astype(BF16)              # [128, H]
    ob1c = np.ascontiguousarray(ob1.reshape(2, 128).T).astype(np.float32)
    ow2s = np.ascontiguousarray(
        ow2.reshape(2, 128, H).transpose(1, 0, 2).reshape(128, 2 * H)).astype(BF16)
    ob2c = np.ascontiguousarray(ob2.reshape(2, 128).T).astype(np.float32)
    ow3s = np.ascontiguousarray(
        ow3.reshape(2, 128, O).transpose(1, 0, 2).reshape(128, 2 * O)).astype(BF16)
    ob3c = np.ascontiguousarray(ob3.reshape(O, 1)).astype(np.float32)
    return dict(
        w1s=w1s, b1c=b1c, w2s=w2s, b2r=b2r,
        ow1s=ow1s, ob1c=ob1c, ow2s=ow2s, ob2c=ob2c, ow3s=ow3s, ob3c=ob3c,
    )


def prep_batch(x, rel_type, e_of_edge, rec_idx):
    """Per-core batched tensors: preT pair-grid gather, rrecrt scatter, xT."""
    bpc = x.shape[0]
    xT = np.ascontiguousarray(x.transpose(0, 2, 1))            # [bpc, F, N]
    preT = np.concatenate(
        [np.repeat(xT, N, axis=2), np.tile(xT, (1, 1, N))], axis=1
    ).astype(BF16)                                              # [bpc, 128, EP]
    rr = np.zeros((bpc, T, EP, N), np.float32)
    rr[:, :, e_of_edge, rec_idx] = rel_type.transpose(0, 2, 1)  # [bpc, T, E]
    rrt = np.ascontiguousarray(
        rr.reshape(bpc, T, NT, 128, N).transpose(0, 3, 1, 2, 4)
        .reshape(bpc, 128, T * NT * N)
    ).astype(BF16)
    return dict(preT=preT, rrt=rrt, xT=xT.astype(BF16))


def kernel(**inputs):
    from concourse.bass_utils import run_bass_kernel_spmd

    f32arrs = {k: np.asarray(v, dtype=np.float32) for k, v in inputs.items()}
    shared = prep_shared(
        f32arrs["w1"], f32arrs["b1"], f32arrs["w2"], f32arrs["b2"],
        f32arrs["ow1"], f32arrs["ob1"], f32arrs["ow2"], f32arrs["ob2"],
        f32arrs["ow3"], f32arrs["ob3"],
    )
    e_of_edge, rec_idx = edge_maps(f32arrs["rel_rec"], f32arrs["rel_send"])
    in_maps = []
    for c in range(NCORES):
        sl = slice(c * BPC, (c + 1) * BPC)
        m = dict(shared)
        m.update(prep_batch(
            f32arrs["x"][sl], f32arrs["rel_type"][sl], e_of_edge, rec_idx))
        in_maps.append(m)

    nc = build_nc(BPC)
    res = run_bass_kernel_spmd(nc, in_maps, list(range(NCORES)))
    # y per core: [BPC, O, N] (transposed) -> full [B, N, O]
    y = np.concatenate([res.results[c]["y"] for c in range(NCORES)], axis=0)
    return np.ascontiguousarray(y.transpose(0, 2, 1)).astype(np.float32)


if __name__ == "__main__":
    # smoke: random inputs, shape check only
    rng = np.random.default_rng(0)
    eye = np.eye(N, dtype=np.float32)
    si, ri = [], []
    for i in range(N):
        for j in range(N):
            if i != j:
                si.append(i)
                ri.append(j)
    inputs = {
        "x": rng.standard_normal((B, N, F), dtype=np.float32),
        "rel_type": rng.random((B, E, T), dtype=np.float32),
        "rel_rec": eye[np.array(ri)],
        "rel_send": eye[np.array(si)],
        "w1": rng.standard_normal((T, 2 * F, H), dtype=np.float32) * 0.1,
        "b1": rng.standard_normal((T, H), dtype=np.float32) * 0.1,
        "w2": rng.standard_normal((T, H, O), dtype=np.float32) * 0.1,
        "b2": rng.standard_normal((T, O), dtype=np.float32) * 0.1,
        "ow1": rng.standard_normal((F + O, H), dtype=np.float32) * 0.1,
        "ob1": rng.standard_normal((H,), dtype=np.float32) * 0.1,
        "ow2": rng.standard_normal((H, H), dtype=np.float32) * 0.1,
        "ob2": rng.standard_normal((H,), dtype=np.float32) * 0.1,
        "ow3": rng.standard_normal((H, O), dtype=np.float32) * 0.1,
        "ob3": rng.standard_normal((O,), dtype=np.float32) * 0.1,
    }
    y = kernel(**inputs)
    print("y", y.shape, y.dtype)


# revision 8
# speedup vs baseline: 1.0257x; 1.0208x over previous
"""Trainium2 Bass kernel for NRI-style GNN decoder (nn_Decoder_58600533787128).

Data-parallel over batch across 8 NeuronCores.  All matmuls run in bf16
(fp32 PSUM accumulation); DMA per batch is tiny (x, rel_type, y only) --
the gather/scatter structure lives in small resident one-hot constants on
the dense (i,j) pair grid (4096 pairs incl. diagonal; diagonal killed by
rel_type=0 there).

Per-core math (bpc=8 batches per core), per batch b:
  pre^T[c,e]   = gather: [x^T S ; x^T R] via matmul(lhsT=x, rhs=one-hots)
  h1^T[h,e]    = relu(W1^T-chunks @ pre^T + b1)        (4 edge types)
  m[e,o]       = h1^T-chunks as lhsT @ W2-chunks + b2   (b2 via ones-matmul)
  msc[e,o]     = relu(m) * rel_type[b,e,t]              (rt>=0 folds into relu)
  agg^T[o,n]  += msc^T... via matmul(lhsT=msc, rhs=rrec-tile)   (accum 32x4)
  y^T[o,n]     = MLP(aug^T = [x^T; agg^T])              (output MLP)
Host transposes y^T back to [n,o].
"""
import sys

sys.path.insert(0, "/opt/trn_rl_repo")

import numpy as np
import ml_dtypes

BF16 = ml_dtypes.bfloat16

B, N, F, H, O, T, E = 64, 64, 64, 256, 64, 4, 4032
EP = N * N         # dense pair grid (i,j), 4096, includes diagonal
NT = EP // 128     # 32 pair tiles of 128
NCORES = 8
BPC = B // NCORES  # batches per core


def build_nc(bpc=BPC, num_devices=NCORES, reps=1):
    import concourse.mybir as mybir
    from concourse import bacc, tile

    f32 = mybir.dt.float32
    bf16 = mybir.dt.bfloat16
    AF = mybir.ActivationFunctionType
    ALU = mybir.AluOpType

    nc = bacc.Bacc(
        "TRN2", target_bir_lowering=False, debug=False, num_devices=num_devices
    )
    x_d = nc.declare_dram_parameter("x", [bpc, N, F], bf16, isOutput=False)
    xT_d = nc.declare_dram_parameter("xT", [bpc, F, N], bf16, isOutput=False)
    rt_d = nc.declare_dram_parameter("rt", [bpc, 128, NT * T], f32, isOutput=False)
    rsT_d = nc.declare_dram_parameter("rsT", [N, EP], bf16, isOutput=False)
    rrT_d = nc.declare_dram_parameter("rrT", [N, EP], bf16, isOutput=False)
    rrec_d = nc.declare_dram_parameter("rrec", [128, NT * N], bf16, isOutput=False)
    w1_d = nc.declare_dram_parameter("w1s", [128, T * H], bf16, isOutput=False)
    b1_d = nc.declare_dram_parameter("b1c", [128, T * 2], f32, isOutput=False)
    w2_d = nc.declare_dram_parameter("w2s", [128, T * 2 * O], bf16, isOutput=False)
    b2_d = nc.declare_dram_parameter("b2r", [1, T * 4 * O], bf16, isOutput=False)
    ow1_d = nc.declare_dram_parameter("ow1s", [128, H], bf16, isOutput=False)
    ob1_d = nc.declare_dram_parameter("ob1c", [128, 2], f32, isOutput=False)
    ow2_d = nc.declare_dram_parameter("ow2s", [128, 2 * H], bf16, isOutput=False)
    ob2_d = nc.declare_dram_parameter("ob2c", [128, 2], f32, isOutput=False)
    ow3_d = nc.declare_dram_parameter("ow3s", [128, 2 * O], bf16, isOutput=False)
    ob3_d = nc.declare_dram_parameter("ob3c", [O, 1], f32, isOutput=False)
    y_d = nc.declare_dram_parameter("y", [bpc, O, N], f32, isOutput=True)

    with tile.TileContext(nc) as tc:
        with (
            tc.tile_pool(name="const", bufs=1) as cpool,
            tc.tile_pool(name="work", bufs=3) as wpool,
            tc.tile_pool(name="h1pool", bufs=4) as hpool,
            tc.tile_pool(name="ppre", bufs=2, space="PSUM") as ppre,
            tc.tile_pool(name="ph1", bufs=3, space="PSUM") as ph1,
            tc.tile_pool(name="pm", bufs=2, space="PSUM") as pm,
            tc.tile_pool(name="pagg", bufs=1, space="PSUM") as pagg,
        ):
            # resident constants (one DMA each; layouts prepped host-side)
            rsT = cpool.tile([N, EP], bf16)
            nc.sync.dma_start(rsT[:], rsT_d[:])
            rrT = cpool.tile([N, EP], bf16)
            nc.sync.dma_start(rrT[:], rrT_d[:])
            rrec = cpool.tile([128, NT * N], bf16)
            nc.sync.dma_start(rrec[:], rrec_d[:])
            w1s = cpool.tile([128, T * H], bf16)
            nc.sync.dma_start(w1s[:], w1_d[:])
            b1c = cpool.tile([128, T * 2], f32)
            nc.sync.dma_start(b1c[:], b1_d[:])
            w2s = cpool.tile([128, T * 2 * O], bf16)
            nc.sync.dma_start(w2s[:], w2_d[:])
            b2r = cpool.tile([1, T * 4 * O], bf16)
            nc.sync.dma_start(b2r[:], b2_d[:])
            ow1s = cpool.tile([128, H], bf16)
            nc.sync.dma_start(ow1s[:], ow1_d[:])
            ob1c = cpool.tile([128, 2], f32)
            nc.sync.dma_start(ob1c[:], ob1_d[:])
            ow2s = cpool.tile([128, 2 * H], bf16)
            nc.sync.dma_start(ow2s[:], ow2_d[:])
            ob2c = cpool.tile([128, 2], f32)
            nc.sync.dma_start(ob2c[:], ob2_d[:])
            ow3s = cpool.tile([128, 2 * O], bf16)
            nc.sync.dma_start(ow3s[:], ow3_d[:])
            ob3c = cpool.tile([O, 1], f32)
            nc.sync.dma_start(ob3c[:], ob3_d[:])
            ones_sb = cpool.tile([1, 128], bf16)
            nc.gpsimd.memset(ones_sb[:], 1.0)

            # element-wise work balancer: PSUM-reading ops go to the less
            # busy of ACT / DVE (GPSIMD cannot touch PSUM on HW); GPSIMD
            # handles SBUF->SBUF scaling.
            busy = [0.0, 0.0]  # ACT, DVE

            def pick(costs):
                e = min(range(2), key=lambda i: busy[i] + costs[i])
                busy[e] += costs[e]
                return e

            def relu_bias(dst, src, bias_col, cols):
                e = pick([cols / 1.2 + 180, cols / 0.96 + 125])
                if e == 0:
                    nc.scalar.activation(dst, src, AF.Relu, bias=bias_col)
                else:
                    nc.vector.tensor_scalar(dst, src, bias_col, 0.0, ALU.add, ALU.max)

            def relu_op(dst, src, cols):
                e = pick([cols / 1.2 + 180, cols / 0.96 + 125])
                if e == 0:
                    nc.scalar.activation(dst, src, AF.Relu)
                else:
                    nc.vector.tensor_scalar(dst, src, 0.0, None, ALU.max)

            def copy_op(dst, src, cols):
                e = pick([cols / 1.2 + 180, cols / 0.96 + 125])
                if e == 0:
                    nc.scalar.activation(dst, src, AF.Copy)
                else:
                    nc.vector.tensor_copy(dst, src)

            import contextlib
            loop_cm = tc.For_i(0, reps, 1) if reps > 1 else contextlib.nullcontext()
            with loop_cm:
              for b in range(bpc):
                x_sb = wpool.tile([N, F], bf16, tag="x_sb")
                nc.sync.dma_start(x_sb[:], x_d[b])
                augT = wpool.tile([128, N], bf16, tag="augT")
                nc.sync.dma_start(augT[0:F, :], xT_d[b])
                rt_sb = wpool.tile([128, NT * T], f32, tag="rt_sb")
                nc.sync.dma_start(rt_sb[:], rt_d[b])

                aggp = pagg.tile([O, N], f32, tag="aggp")
                for st in range(8):
                    e0 = st * 512
                    # gather: pre^T = [senders^T ; receivers^T] for 512 pairs
                    prep = ppre.tile([128, 512], f32, tag="prep")
                    nc.tensor.matmul(
                        prep[0:64, :], x_sb[:], rsT[:, e0:e0 + 512],
                        start=True, stop=True,
                    )
                    nc.tensor.matmul(
                        prep[64:128, :], x_sb[:], rrT[:, e0:e0 + 512],
                        start=True, stop=True,
                    )
                    preT = wpool.tile([128, 512], bf16, tag="preT")
                    copy_op(preT[:], prep[:], 512)

                    for t in range(T):
                        h1s = hpool.tile([128, 2 * 512], bf16, tag="h1s")
                        for hc in range(2):
                            h1p = ph1.tile([128, 512], f32, tag="h1p")
                            nc.tensor.matmul(
                                h1p[:],
                                w1s[:, t * H + hc * 128: t * H + (hc + 1) * 128],
                                preT[:],
                                start=True, stop=True,
                            )
                            relu_bias(
                                h1s[:, hc * 512:(hc + 1) * 512], h1p[:],
                                b1c[:, t * 2 + hc: t * 2 + hc + 1], 512,
                            )
                        # layer2: 4 psum-subtiles of 128 pairs in one bank
                        mp = pm.tile([128, 4 * O], f32, tag="mp")
                        # preload b2 broadcast over all 128 pair-partitions
                        nc.tensor.matmul(
                            mp[:], ones_sb[:], b2r[0:1, t * 4 * O:(t + 1) * 4 * O],
                            start=True, stop=False, skip_group_check=True,
                        )
                        for sub in range(4):
                            for kc in range(2):
                                nc.tensor.matmul(
                                    mp[:, sub * O:(sub + 1) * O],
                                    h1s[:, kc * 512 + sub * 128: kc * 512 + (sub + 1) * 128],
                                    w2s[:, (t * 2 + kc) * O:(t * 2 + kc + 1) * O],
                                    start=False, stop=(sub == 3 and kc == 1),
                                    skip_group_check=True,
                                )
                        mr = hpool.tile([128, 4 * O], bf16, tag="mr")
                        relu_op(mr[:], mp[:], 256)
                        msc = hpool.tile([128, 4 * O], bf16, tag="msc")
                        for sub in range(4):
                            et = st * 4 + sub
                            # relu(m) * rt  (rt >= 0, diagonal pairs have rt=0)
                            nc.gpsimd.tensor_scalar(
                                msc[:, sub * O:(sub + 1) * O],
                                mr[:, sub * O:(sub + 1) * O],
                                rt_sb[:, et * T + t: et * T + t + 1], None,
                                ALU.mult,
                            )
                        for sub in range(4):
                            et = st * 4 + sub
                            nc.tensor.matmul(
                                aggp[:], msc[:, sub * O:(sub + 1) * O],
                                rrec[:, et * N:(et + 1) * N],
                                start=(st == 0 and t == 0 and sub == 0),
                                stop=(st == 7 and t == T - 1 and sub == 3),
                                skip_group_check=True,
                            )

                # output MLP on aug^T = [x^T ; agg^T]
                nc.vector.tensor_copy(augT[F:128, :], aggp[:])
                f1 = wpool.tile([128, 2 * N], bf16, tag="f1")
                for mc in range(2):
                    fp = pm.tile([128, 4 * O], f32, tag="mp")
                    nc.tensor.matmul(
                        fp[:, 0:N], ow1s[:, mc * 128:(mc + 1) * 128], augT[:],
                        start=True, stop=True,
                    )
                    nc.scalar.activation(
                        f1[:, mc * N:(mc + 1) * N], fp[:, 0:N], AF.Relu,
                        bias=ob1c[:, mc:mc + 1],
                    )
                f2 = wpool.tile([128, 2 * N], bf16, tag="f2")
                for mc in range(2):
                    fp = pm.tile([128, 4 * O], f32, tag="mp")
                    for kc in range(2):
                        nc.tensor.matmul(
                            fp[:, 0:N],
                            ow2s[:, kc * H + mc * 128: kc * H + (mc + 1) * 128],
                            f1[:, kc * N:(kc + 1) * N],
                            start=(kc == 0), stop=(kc == 1),
                        )
                    nc.scalar.activation(
                        f2[:, mc * N:(mc + 1) * N], fp[:, 0:N], AF.Relu,
                        bias=ob2c[:, mc:mc + 1],
                    )
                op = pm.tile([128, 4 * O], f32, tag="mp")
                for kc in range(2):
                    nc.tensor.matmul(
                        op[0:O, 0:N], ow3s[:, kc * O:(kc + 1) * O],
                        f2[:, kc * N:(kc + 1) * N],
                        start=(kc == 0), stop=(kc == 1),
                    )
                yb = wpool.tile([O, N], f32, tag="yb")
                nc.vector.tensor_scalar(
                    yb[:], op[0:O, 0:N], ob3c[:, 0:1], None, ALU.add
                )
                nc.sync.dma_start(y_d[b], yb[:])

    nc.compile()
    return nc


def edge_maps(rel_rec, rel_send):
    """Pair-grid index and receiver node for each of the E directed edges."""
    send_idx = np.argmax(rel_send, axis=1).astype(np.int64)  # [E]
    rec_idx = np.argmax(rel_rec, axis=1).astype(np.int64)    # [E]
    return send_idx * N + rec_idx, rec_idx


def prep_shared(w1, b1, w2, b2, ow1, ob1, ow2, ob2, ow3, ob3):
    """Host-side layout prep for the replicated weights (bf16) + one-hots."""
    # pair-grid one-hot structure: pair e = i*N + j
    i_of = np.repeat(np.arange(N), N)
    j_of = np.tile(np.arange(N), N)
    rsT = np.zeros((N, EP), np.float32)
    rsT[i_of, np.arange(EP)] = 1.0            # senders^T one-hot
    rrT = np.zeros((N, EP), np.float32)
    rrT[j_of, np.arange(EP)] = 1.0            # receivers^T one-hot
    rrec_pad = np.zeros((EP, N), np.float32)
    rrec_pad[np.arange(EP), j_of] = 1.0       # receiver scatter (diag killed by rt)
    rrec_t = np.ascontiguousarray(
        rrec_pad.reshape(NT, 128, N).transpose(1, 0, 2).reshape(128, NT * N)
    )
    w1s = np.ascontiguousarray(
        w1.transpose(1, 0, 2).reshape(2 * F, T * H)).astype(BF16)
    b1c = np.ascontiguousarray(
        b1.reshape(T, 2, 128).transpose(2, 0, 1).reshape(128, T * 2)
    ).astype(np.float32)
    w2s = np.ascontiguousarray(
        w2.reshape(T, 2, 128, O).transpose(2, 0, 1, 3).reshape(128, T * 2 * O)
    ).astype(BF16)
    b2r = np.ascontiguousarray(
        np.tile(b2, (1, 4)).reshape(1, T * 4 * O)).astype(BF16)
    ow1s = np.ascontiguousarray(ow1).astype(BF16)              # [128, H]
    ob1c = np.ascontiguousarray(ob1.reshape(2, 128).T).astype(np.float32)
    ow2s = np.ascontiguousarray(
        ow2.reshape(2, 128, H).transpose(1, 0, 2).reshape(128, 2 * H)).astype(BF16)
    ob2c = np.ascontiguousarray(ob2.reshape(2, 128).T).astype(np.float32)
    ow3s = np.ascontiguousarray(
        ow3.reshape(2, 128, O).transpose(1, 0, 2).reshape(128, 2 * O)).astype(BF16)
    ob3c = np.ascontiguousarray(ob3.reshape(O, 1)).astype(np.float32)
    return dict(
        rsT=rsT.astype(BF16), rrT=rrT.astype(BF16), rrec=rrec_t.astype(BF16),
        w1s=w1s, b1c=b1c, w2s=w2s, b2r=b2r,
        ow1s=ow1s, ob1c=ob1c, ow2s=ow2s, ob2c=ob2c, ow3s=ow3s, ob3c=ob3c,
    )


def prep_batch(x, rel_type, e_of_edge, rec_idx):
    """Per-core batched tensors: x, xT, rel_type scattered to the pair grid."""
    bpc = x.shape[0]
    xT = np.ascontiguousarray(x.transpose(0, 2, 1))            # [bpc, F, N]
    rt_pad = np.zeros((bpc, EP, T), np.float32)
    rt_pad[:, e_of_edge, :] = rel_type                          # diag stays 0
    # [bpc, EP, T] -> [bpc, 128, NT*T] : col et*T+t is rel_type for tile et
    rt_t = np.ascontiguousarray(
        rt_pad.reshape(bpc, NT, 128, T).transpose(0, 2, 1, 3).reshape(bpc, 128, NT * T)
    )
    return dict(x=np.ascontiguousarray(x).astype(BF16), xT=xT.astype(BF16),
                rt=rt_t)


def kernel(**inputs):
    from concourse.bass_utils import run_bass_kernel_spmd

    f32arrs = {k: np.asarray(v, dtype=np.float32) for k, v in inputs.items()}
    shared = prep_shared(
        f32arrs["w1"], f32arrs["b1"], f32arrs["w2"], f32arrs["b2"],
        f32arrs["ow1"], f32arrs["ob1"], f32arrs["ow2"], f32arrs["ob2"],
        f32arrs["ow3"], f32arrs["ob3"],
    )
    e_of_edge, rec_idx = edge_maps(f32arrs["rel_rec"], f32arrs["rel_send"])
    in_maps = []
    for c in range(NCORES):
        sl = slice(c * BPC, (c + 1) * BPC)
        m = dict(shared)
        m.update(prep_batch(
            f32arrs["x"][sl], f32arrs["rel_type"][sl], e_of_edge, rec_idx))
        in_maps.append(m)

    nc = build_nc(BPC)
    res = run_bass_kernel_spmd(nc, in_maps, list(range(NCORES)))
    # y per core: [BPC, O, N] (transposed) -> full [B, N, O]
    y = np.concatenate([res.results[c]["y"] for c in range(NCORES)], axis=0)
    return np.ascontiguousarray(y.transpose(0, 2, 1)).astype(np.float32)


if __name__ == "__main__":
    # smoke: random inputs, shape check only
    rng = np.random.default_rng(0)
    eye = np.eye(N, dtype=np.float32)
    si, ri = [], []
    for i in range(N):
        for j in range(N):
            if i != j:
                si.append(i)
                ri.append(j)
    inputs = {
        "x": rng.standard_normal((B, N, F), dtype=np.float32),
        "rel_type": rng.random((B, E, T), dtype=np.float32),
        "rel_rec": eye[np.array(ri)],
        "rel_send": eye[np.array(si)],
        "w1": rng.standard_normal((T, 2 * F, H), dtype=np.float32) * 0.1,
        "b1": rng.standard_normal((T, H), dtype=np.float32) * 0.1,
        "w2": rng.standard_normal((T, H, O), dtype=np.float32) * 0.1,
        "b2": rng.standard_normal((T, O), dtype=np.float32) * 0.1,
        "ow1": rng.standard_normal((F + O, H), dtype=np.float32) * 0.1,
        "ob1": rng.standard_normal((H,), dtype=np.float32) * 0.1,
        "ow2": rng.standard_normal((H, H), dtype=np.float32) * 0.1,
        "ob2": rng.standard_normal((H,), dtype=np.float32) * 0.1,
        "ow3": rng.standard_normal((H, O), dtype=np.float32) * 0.1,
        "ob3": rng.standard_normal((O,), dtype=np.float32) * 0.1,
    }
    y = kernel(**inputs)
    print("y", y.shape, y.dtype)


# revision 12
# speedup vs baseline: 1.6078x; 1.5674x over previous
"""Trainium2 Bass kernel for NRI-style GNN decoder (nn_Decoder_58600533787128).

Data-parallel over batch across 8 NeuronCores.  All matmuls are bf16 with
free dim 512 (small-free matmuls measured ~10x slower per instruction on
HW, so layer2 runs transposed and the edge->node aggregation uses the
dense pair-grid structure as a strided DVE reduction instead of one-hot
matmuls).

Pair grid: e = i*64 + j (4096 pairs incl. diagonal; diagonal killed by
rel_type=0).  Per batch:
  pre^T[f,e]  = gather [x^T S; x^T R] via one-hot matmuls     (PE, free 512)
  h1^T[h,e]   = relu(W1^T @ pre^T + b1)                       (ACT/DVE)
  mT[o,e]     = W2-chunks^T @ h1^T-chunks                     (PE, free 512)
  r2[o,e]     = relu(mT + b2)          b2 per-partition bias  (ACT/DVE)
  s[o,e]      = r2 * rt_bcast[e]       rt via rank-1 matmul   (DVE)
  agg[o,j]   += sum_i s[o, i*64+j]     strided segment reduce (DVE) + add (Pool)
  y^T[o,n]    = MLP([x^T ; agg^T])                            (PE + ACT)
All [o,*] tiles live on partitions 64..127 so lanes stay aligned with the
aug^T layout ([x^T on 0..63 ; agg^T on 64..127]).
"""
import sys

sys.path.insert(0, "/opt/trn_rl_repo")

import numpy as np
import ml_dtypes

BF16 = ml_dtypes.bfloat16

B, N, F, H, O, T, E = 64, 64, 64, 256, 64, 4, 4032
EP = N * N         # dense pair grid (i,j), 4096, includes diagonal
NST = 8            # supertiles of 512 pairs
NCORES = 8
BPC = B // NCORES  # batches per core


def build_nc(bpc=BPC, num_devices=NCORES, reps=1):
    import concourse.mybir as mybir
    from concourse import bacc, tile

    f32 = mybir.dt.float32
    bf16 = mybir.dt.bfloat16
    AF = mybir.ActivationFunctionType
    ALU = mybir.AluOpType
    AX = mybir.AxisListType

    nc = bacc.Bacc(
        "TRN2", target_bir_lowering=False, debug=False, num_devices=num_devices
    )
    x_d = nc.declare_dram_parameter("x", [bpc, N, F], bf16, isOutput=False)
    xT_d = nc.declare_dram_parameter("xT", [bpc, F, N], bf16, isOutput=False)
    rt_d = nc.declare_dram_parameter("rt32", [bpc, T * NST, 512], bf16, isOutput=False)
    rsT_d = nc.declare_dram_parameter("rsT", [N, EP], bf16, isOutput=False)
    rrT_d = nc.declare_dram_parameter("rrT", [N, EP], bf16, isOutput=False)
    w1_d = nc.declare_dram_parameter("w1s", [128, T * H], bf16, isOutput=False)
    b1_d = nc.declare_dram_parameter("b1c", [128, T * 2], f32, isOutput=False)
    w2_d = nc.declare_dram_parameter("w2s", [128, T * 2 * O], bf16, isOutput=False)
    b2_d = nc.declare_dram_parameter("b2cT", [128, T], f32, isOutput=False)
    ow1_d = nc.declare_dram_parameter("ow1s", [128, H], bf16, isOutput=False)
    ob1_d = nc.declare_dram_parameter("ob1c", [128, 2], f32, isOutput=False)
    ow2_d = nc.declare_dram_parameter("ow2s", [128, 2 * H], bf16, isOutput=False)
    ob2_d = nc.declare_dram_parameter("ob2c", [128, 2], f32, isOutput=False)
    ow3_d = nc.declare_dram_parameter("ow3s", [128, 2 * O], bf16, isOutput=False)
    ob3_d = nc.declare_dram_parameter("ob3c", [O, 1], f32, isOutput=False)
    y_d = nc.declare_dram_parameter("y", [bpc, O, N], f32, isOutput=True)

    with tile.TileContext(nc) as tc:
        with (
            tc.tile_pool(name="const", bufs=1) as cpool,
            tc.tile_pool(name="work", bufs=3) as wpool,
            tc.tile_pool(name="h1pool", bufs=4) as hpool,
            tc.tile_pool(name="spool", bufs=4) as spool,
            tc.tile_pool(name="ppre", bufs=1, space="PSUM") as ppre,
            tc.tile_pool(name="ph1", bufs=3, space="PSUM") as ph1,
            tc.tile_pool(name="pmT", bufs=2, space="PSUM") as pmT,
            tc.tile_pool(name="prt", bufs=2, space="PSUM") as prt,
        ):
            # resident constants (one DMA each; layouts prepped host-side)
            rsT = cpool.tile([N, EP], bf16)
            nc.sync.dma_start(rsT[:], rsT_d[:])
            rrT = cpool.tile([N, EP], bf16)
            nc.sync.dma_start(rrT[:], rrT_d[:])
            w1s = cpool.tile([128, T * H], bf16)
            nc.sync.dma_start(w1s[:], w1_d[:])
            b1c = cpool.tile([128, T * 2], f32)
            nc.sync.dma_start(b1c[:], b1_d[:])
            w2s = cpool.tile([128, T * 2 * O], bf16)
            nc.sync.dma_start(w2s[:], w2_d[:])
            b2cT = cpool.tile([128, T], f32)
            nc.sync.dma_start(b2cT[:], b2_d[:])
            ow1s = cpool.tile([128, H], bf16)
            nc.sync.dma_start(ow1s[:], ow1_d[:])
            ob1c = cpool.tile([128, 2], f32)
            nc.sync.dma_start(ob1c[:], ob1_d[:])
            ow2s = cpool.tile([128, 2 * H], bf16)
            nc.sync.dma_start(ow2s[:], ow2_d[:])
            ob2c = cpool.tile([128, 2], f32)
            nc.sync.dma_start(ob2c[:], ob2_d[:])
            ow3s = cpool.tile([128, 2 * O], bf16)
            nc.sync.dma_start(ow3s[:], ow3_d[:])
            ob3c = cpool.tile([O, 1], f32)
            nc.sync.dma_start(ob3c[:], ob3_d[:])
            # sel32[k, row*64+m] = 1 iff k == row: rank-1 row selector for the
            # rt broadcast matmul (base partitions must be 0/32/64)
            sel32 = cpool.tile([T * NST, T * NST * N], bf16)
            sel32_d = nc.declare_dram_parameter(
                "sel32", [T * NST, T * NST * N], bf16, isOutput=False)
            nc.sync.dma_start(sel32[:], sel32_d[:])

            # ACT / DVE balancer for PSUM-reading element ops
            busy = [0.0, 0.0]

            def pick(costs):
                e = min(range(2), key=lambda i: busy[i] + costs[i])
                busy[e] += costs[e]
                return e

            def relu_bias(dst, src, bias_col, cols):
                e = pick([cols / 1.2 + 180, cols / 0.96 + 125])
                if e == 0:
                    nc.scalar.activation(dst, src, AF.Relu, bias=bias_col)
                else:
                    nc.vector.tensor_scalar(dst, src, bias_col, 0.0, ALU.add, ALU.max)

            def copy_op(dst, src, cols):
                e = pick([cols / 1.2 + 180, cols / 0.96 + 125])
                if e == 0:
                    nc.scalar.activation(dst, src, AF.Copy)
                else:
                    nc.vector.tensor_copy(dst, src)

            import contextlib
            loop_cm = tc.For_i(0, reps, 1) if reps > 1 else contextlib.nullcontext()
            with loop_cm:
              for b in range(bpc):
                x_sb = wpool.tile([N, F], bf16, tag="x_sb")
                nc.sync.dma_start(x_sb[:], x_d[b])
                augT = wpool.tile([128, N], bf16, tag="augT")
                nc.sync.dma_start(augT[0:F, :], xT_d[b])
                rt32 = wpool.tile([T * NST, 512], bf16, tag="rt32")
                nc.sync.dma_start(rt32[:], rt_d[b])

                aggT = wpool.tile([128, N], f32, tag="aggT")
                first = True
                for st in range(NST):
                    e0 = st * 512
                    # gather: pre^T = [senders^T ; receivers^T] for 512 pairs
                    prep = ppre.tile([128, 512], f32, tag="prep")
                    nc.tensor.matmul(
                        prep[0:64, :], x_sb[:], rsT[:, e0:e0 + 512],
                        start=True, stop=True,
                    )
                    nc.tensor.matmul(
                        prep[64:128, :], x_sb[:], rrT[:, e0:e0 + 512],
                        start=True, stop=True,
                    )
                    preT = wpool.tile([128, 512], bf16, tag="preT")
                    copy_op(preT[:], prep[:], 512)

                    for t in range(T):
                        h1s = hpool.tile([128, 2 * 512], bf16, tag="h1s")
                        for hc in range(2):
                            h1p = ph1.tile([128, 512], f32, tag="h1p")
                            nc.tensor.matmul(
                                h1p[:],
                                w1s[:, t * H + hc * 128: t * H + (hc + 1) * 128],
                                preT[:],
                                start=True, stop=True,
                            )
                            relu_bias(
                                h1s[:, hc * 512:(hc + 1) * 512], h1p[:],
                                b1c[:, t * 2 + hc: t * 2 + hc + 1], 512,
                            )
                        # layer2 transposed: mT[o, e] on partitions 64..127
                        mTp = pmT.tile([128, 512], f32, tag="mTp")
                        for kc in range(2):
                            nc.tensor.matmul(
                                mTp[64:128, :],
                                w2s[:, (t * 2 + kc) * O:(t * 2 + kc + 1) * O],
                                h1s[:, kc * 512:(kc + 1) * 512],
                                start=(kc == 0), stop=(kc == 1),
                            )
                        # rt broadcast across O-partitions via rank-1 matmul
                        rtp = prt.tile([128, 512], f32, tag="rtp")
                        row = t * NST + st
                        nc.tensor.matmul(
                            rtp[64:128, :],
                            sel32[:, row * N:(row + 1) * N],
                            rt32[:],
                            start=True, stop=True,
                        )
                        r2 = spool.tile([128, 512], bf16, tag="r2")
                        relu_bias(
                            r2[64:128, :], mTp[64:128, :],
                            b2cT[64:128, t:t + 1], 512,
                        )
                        s = spool.tile([128, 512], bf16, tag="s")
                        nc.vector.tensor_tensor(
                            s[64:128, :], r2[64:128, :], rtp[64:128, :], ALU.mult
                        )
                        red = spool.tile([128, N], f32, tag="red")
                        nc.vector.tensor_reduce(
                            red[64:128, :],
                            s[64:128, :].rearrange("p (i j) -> p j i", i=NST, j=N),
                            AX.X, ALU.add,
                        )
                        if first:
                            nc.gpsimd.tensor_copy(aggT[64:128, :], red[64:128, :])
                            first = False
                        else:
                            nc.gpsimd.tensor_tensor(
                                aggT[64:128, :], aggT[64:128, :], red[64:128, :],
                                ALU.add,
                            )

                # output MLP on aug^T = [x^T ; agg^T]
                nc.gpsimd.tensor_copy(augT[F:128, :], aggT[64:128, :])
                f1 = wpool.tile([128, 2 * N], bf16, tag="f1")
                for mc in range(2):
                    fp = pmT.tile([128, 512], f32, tag="mTp")
                    nc.tensor.matmul(
                        fp[:, 0:N], ow1s[:, mc * 128:(mc + 1) * 128], augT[:],
                        start=True, stop=True,
                    )
                    nc.scalar.activation(
                        f1[:, mc * N:(mc + 1) * N], fp[:, 0:N], AF.Relu,
                        bias=ob1c[:, mc:mc + 1],
                    )
                f2 = wpool.tile([128, 2 * N], bf16, tag="f2")
                for mc in range(2):
                    fp = pmT.tile([128, 512], f32, tag="mTp")
                    for kc in range(2):
                        nc.tensor.matmul(
                            fp[:, 0:N],
                            ow2s[:, kc * H + mc * 128: kc * H + (mc + 1) * 128],
                            f1[:, kc * N:(kc + 1) * N],
                            start=(kc == 0), stop=(kc == 1),
                        )
                    nc.scalar.activation(
                        f2[:, mc * N:(mc + 1) * N], fp[:, 0:N], AF.Relu,
                        bias=ob2c[:, mc:mc + 1],
                    )
                op = pmT.tile([128, 512], f32, tag="mTp")
                for kc in range(2):
                    nc.tensor.matmul(
                        op[0:O, 0:N], ow3s[:, kc * O:(kc + 1) * O],
                        f2[:, kc * N:(kc + 1) * N],
                        start=(kc == 0), stop=(kc == 1),
                    )
                yb = wpool.tile([O, N], f32, tag="yb")
                nc.vector.tensor_scalar(
                    yb[:], op[0:O, 0:N], ob3c[:, 0:1], None, ALU.add
                )
                nc.sync.dma_start(y_d[b], yb[:])

    nc.compile()
    return nc


def edge_maps(rel_rec, rel_send):
    """Pair-grid index and receiver node for each of the E directed edges."""
    send_idx = np.argmax(rel_send, axis=1).astype(np.int64)  # [E]
    rec_idx = np.argmax(rel_rec, axis=1).astype(np.int64)    # [E]
    return send_idx * N + rec_idx, rec_idx


def prep_shared(w1, b1, w2, b2, ow1, ob1, ow2, ob2, ow3, ob3):
    """Host-side layout prep for the replicated weights (bf16) + one-hots."""
    i_of = np.repeat(np.arange(N), N)
    j_of = np.tile(np.arange(N), N)
    rsT = np.zeros((N, EP), np.float32)
    rsT[i_of, np.arange(EP)] = 1.0            # senders^T one-hot
    rrT = np.zeros((N, EP), np.float32)
    rrT[j_of, np.arange(EP)] = 1.0            # receivers^T one-hot
    w1s = np.ascontiguousarray(
        w1.transpose(1, 0, 2).reshape(2 * F, T * H)).astype(BF16)
    b1c = np.ascontiguousarray(
        b1.reshape(T, 2, 128).transpose(2, 0, 1).reshape(128, T * 2)
    ).astype(np.float32)
    w2s = np.ascontiguousarray(
        w2.reshape(T, 2, 128, O).transpose(2, 0, 1, 3).reshape(128, T * 2 * O)
    ).astype(BF16)
    b2cT = np.zeros((128, T), np.float32)
    b2cT[64:128, :] = b2.T                    # bias on partitions 64..127
    ow1s = np.ascontiguousarray(ow1).astype(BF16)              # [128, H]
    ob1c = np.ascontiguousarray(ob1.reshape(2, 128).T).astype(np.float32)
    ow2s = np.ascontiguousarray(
        ow2.reshape(2, 128, H).transpose(1, 0, 2).reshape(128, 2 * H)).astype(BF16)
    ob2c = np.ascontiguousarray(ob2.reshape(2, 128).T).astype(np.float32)
    ow3s = np.ascontiguousarray(
        ow3.reshape(2, 128, O).transpose(1, 0, 2).reshape(128, 2 * O)).astype(BF16)
    ob3c = np.ascontiguousarray(ob3.reshape(O, 1)).astype(np.float32)
    R = T * NST
    sel32 = np.zeros((R, R * N), np.float32)
    for r in range(R):
        sel32[r, r * N:(r + 1) * N] = 1.0
    return dict(
        rsT=rsT.astype(BF16), rrT=rrT.astype(BF16),
        w1s=w1s, b1c=b1c, w2s=w2s, b2cT=b2cT, sel32=sel32.astype(BF16),
        ow1s=ow1s, ob1c=ob1c, ow2s=ow2s, ob2c=ob2c, ow3s=ow3s, ob3c=ob3c,
    )


def prep_batch(x, rel_type, e_of_edge, rec_idx):
    """Per-core batched tensors: x, xT, rel_type as [T*NST, 512] rows."""
    bpc = x.shape[0]
    xT = np.ascontiguousarray(x.transpose(0, 2, 1))            # [bpc, F, N]
    rt_pad = np.zeros((bpc, EP, T), np.float32)
    rt_pad[:, e_of_edge, :] = rel_type                          # diag stays 0
    # row t*NST+st holds rel_type for pairs [st*512, (st+1)*512), type t
    rt32 = np.ascontiguousarray(
        rt_pad.reshape(bpc, NST, 512, T).transpose(0, 3, 1, 2).reshape(
            bpc, T * NST, 512)
    ).astype(BF16)
    return dict(x=np.ascontiguousarray(x).astype(BF16), xT=xT.astype(BF16),
                rt32=rt32)


def kernel(**inputs):
    from concourse.bass_utils import run_bass_kernel_spmd

    f32arrs = {k: np.asarray(v, dtype=np.float32) for k, v in inputs.items()}
    shared = prep_shared(
        f32arrs["w1"], f32arrs["b1"], f32arrs["w2"], f32arrs["b2"],
        f32arrs["ow1"], f32arrs["ob1"], f32arrs["ow2"], f32arrs["ob2"],
        f32arrs["ow3"], f32arrs["ob3"],
    )
    e_of_edge, rec_idx = edge_maps(f32arrs["rel_rec"], f32arrs["rel_send"])
    in_maps = []
    for c in range(NCORES):
        sl = slice(c * BPC, (c + 1) * BPC)
        m = dict(shared)
        m.update(prep_batch(
            f32arrs["x"][sl], f32arrs["rel_type"][sl], e_of_edge, rec_idx))
        in_maps.append(m)

    nc = build_nc(BPC)
    res = run_bass_kernel_spmd(nc, in_maps, list(range(NCORES)))
    # y per core: [BPC, O, N] (transposed) -> full [B, N, O]
    y = np.concatenate([res.results[c]["y"] for c in range(NCORES)], axis=0)
    return np.ascontiguousarray(y.transpose(0, 2, 1)).astype(np.float32)


if __name__ == "__main__":
    # smoke: random inputs, shape check only
    rng = np.random.default_rng(0)
    eye = np.eye(N, dtype=np.float32)
    si, ri = [], []
    for i in range(N):
        for j in range(N):
            if i != j:
                si.append(i)
                ri.append(j)
    inputs = {
        "x": rng.standard_normal((B, N, F), dtype=np.float32),
        "rel_type": rng.random((B, E, T), dtype=np.float32),
        "rel_rec": eye[np.array(ri)],
        "rel_send": eye[np.array(si)],
        "w1": rng.standard_normal((T, 2 * F, H), dtype=np.float32) * 0.1,
        "b1": rng.standard_normal((T, H), dtype=np.float32) * 0.1,
        "w2": rng.standard_normal((T, H, O), dtype=np.float32) * 0.1,
        "b2": rng.standard_normal((T, O), dtype=np.float32) * 0.1,
        "ow1": rng.standard_normal((F + O, H), dtype=np.float32) * 0.1,
        "ob1": rng.standard_normal((H,), dtype=np.float32) * 0.1,
        "ow2": rng.standard_normal((H, H), dtype=np.float32) * 0.1,
        "ob2": rng.standard_normal((H,), dtype=np.float32) * 0.1,
        "ow3": rng.standard_normal((H, O), dtype=np.float32) * 0.1,
        "ob3": rng.standard_normal((O,), dtype=np.float32) * 0.1,
    }
    y = kernel(**inputs)
    print("y", y.shape, y.dtype)


# revision 13
# speedup vs baseline: 3.1512x; 1.9599x over previous
"""Trainium2 Bass kernel for NRI-style GNN decoder (nn_Decoder_58600533787128).

Data-parallel over batch across 8 NeuronCores.  All matmuls are bf16 with
free dim >= 512 (small-free matmuls measured ~10x slower per instruction
on HW): layer2 runs transposed with two edge types packed into the 128
partitions, the edge->node aggregation is a strided DVE reduction over the
receiver-major dense pair grid, and the output MLP is batched over all 8
per-core batches in one free-512 pass.

Pair grid: e = j*64 + i (receiver-major, 4096 pairs incl. diagonal;
diagonal killed by rel_type=0).  Supertile st covers receivers
[8st, 8st+8).  Per batch:
  pre^T[f,e]   = gather [x^T S; x^T R] via one-hot matmuls    (PE)
  h1^T[h,e]    = relu(W1^T @ pre^T + b1)  per type            (ACT/DVE)
  mT[o2,e]     = W2^T @ h1^T   2 types on partition halves    (PE)
  r2           = relu(mT + b2-packed)                         (ACT/DVE)
  s            = r2 * rt-packed (rank-1 selector matmul)      (DVE)
  red[o2,8]    = sum_i s[o2, j*64+i]  contiguous reduce       (DVE)
  aggT2[o2,j] += red   (types 0+2 on top half, 1+3 bottom)    (Pool)
  augA[.,b]    = [x^T ; agg-bottom], aggA_top separate        (Pool)
Then one batched MLP over augA [128, 512] with an extra accumulating
matmul folding aggA_top in through ow1's agg rows.
"""
import sys

sys.path.insert(0, "/opt/trn_rl_repo")

import numpy as np
import ml_dtypes

BF16 = ml_dtypes.bfloat16

B, N, F, H, O, T, E = 64, 64, 64, 256, 64, 4, 4032
EP = N * N         # dense pair grid (j,i), 4096, includes diagonal
NST = 8            # supertiles of 512 pairs (8 receivers each)
NCORES = 8
BPC = B // NCORES  # batches per core


def build_nc(bpc=BPC, num_devices=NCORES, reps=1):
    import concourse.mybir as mybir
    from concourse import bacc, tile

    f32 = mybir.dt.float32
    bf16 = mybir.dt.bfloat16
    AF = mybir.ActivationFunctionType
    ALU = mybir.AluOpType
    AX = mybir.AxisListType

    nc = bacc.Bacc(
        "TRN2", target_bir_lowering=False, debug=False, num_devices=num_devices
    )
    x_d = nc.declare_dram_parameter("x", [bpc, N, F], bf16, isOutput=False)
    xT_d = nc.declare_dram_parameter("xT", [bpc, F, N], bf16, isOutput=False)
    rt_d = nc.declare_dram_parameter("rt32", [bpc, T * NST, 512], bf16, isOutput=False)
    rsT_d = nc.declare_dram_parameter("rsT", [N, EP], bf16, isOutput=False)
    rrT_d = nc.declare_dram_parameter("rrT", [N, EP], bf16, isOutput=False)
    sel2_d = nc.declare_dram_parameter(
        "sel2", [T * NST, 2 * NST * 128], bf16, isOutput=False)
    w1_d = nc.declare_dram_parameter("w1s", [128, T * H], bf16, isOutput=False)
    b1_d = nc.declare_dram_parameter("b1c", [128, T * 2], f32, isOutput=False)
    w2_d = nc.declare_dram_parameter("w2s", [128, T * 2 * O], bf16, isOutput=False)
    b2_d = nc.declare_dram_parameter("b2p", [128, 2], f32, isOutput=False)
    ow1_d = nc.declare_dram_parameter("ow1s", [128, H], bf16, isOutput=False)
    ow1b_d = nc.declare_dram_parameter("ow1b0", [N, H], bf16, isOutput=False)
    ob1_d = nc.declare_dram_parameter("ob1c", [128, 2], f32, isOutput=False)
    ow2_d = nc.declare_dram_parameter("ow2s", [128, 2 * H], bf16, isOutput=False)
    ob2_d = nc.declare_dram_parameter("ob2c", [128, 2], f32, isOutput=False)
    ow3_d = nc.declare_dram_parameter("ow3s", [128, 2 * O], bf16, isOutput=False)
    ob3_d = nc.declare_dram_parameter("ob3c", [O, 1], f32, isOutput=False)
    y_d = nc.declare_dram_parameter("y", [O, bpc * N], f32, isOutput=True)

    with tile.TileContext(nc) as tc:
        with (
            tc.tile_pool(name="const", bufs=1) as cpool,
            tc.tile_pool(name="work", bufs=3) as wpool,
            tc.tile_pool(name="h1pool", bufs=4) as hpool,
            tc.tile_pool(name="spool", bufs=4) as spool,
            tc.tile_pool(name="ppre", bufs=1, space="PSUM") as ppre,
            tc.tile_pool(name="ph1", bufs=3, space="PSUM") as ph1,
            tc.tile_pool(name="pmT", bufs=2, space="PSUM") as pmT,
            tc.tile_pool(name="prt", bufs=2, space="PSUM") as prt,
        ):
            # resident constants (one DMA each; layouts prepped host-side)
            rsT = cpool.tile([N, EP], bf16)
            nc.sync.dma_start(rsT[:], rsT_d[:])
            rrT = cpool.tile([N, EP], bf16)
            nc.sync.dma_start(rrT[:], rrT_d[:])
            sel2 = cpool.tile([T * NST, 2 * NST * 128], bf16)
            nc.sync.dma_start(sel2[:], sel2_d[:])
            w1s = cpool.tile([128, T * H], bf16)
            nc.sync.dma_start(w1s[:], w1_d[:])
            b1c = cpool.tile([128, T * 2], f32)
            nc.sync.dma_start(b1c[:], b1_d[:])
            w2s = cpool.tile([128, T * 2 * O], bf16)
            nc.sync.dma_start(w2s[:], w2_d[:])
            b2p = cpool.tile([128, 2], f32)
            nc.sync.dma_start(b2p[:], b2_d[:])
            ow1s = cpool.tile([128, H], bf16)
            nc.sync.dma_start(ow1s[:], ow1_d[:])
            ow1b0 = cpool.tile([N, H], bf16)
            nc.sync.dma_start(ow1b0[:], ow1b_d[:])
            ob1c = cpool.tile([128, 2], f32)
            nc.sync.dma_start(ob1c[:], ob1_d[:])
            ow2s = cpool.tile([128, 2 * H], bf16)
            nc.sync.dma_start(ow2s[:], ow2_d[:])
            ob2c = cpool.tile([128, 2], f32)
            nc.sync.dma_start(ob2c[:], ob2_d[:])
            ow3s = cpool.tile([128, 2 * O], bf16)
            nc.sync.dma_start(ow3s[:], ow3_d[:])
            ob3c = cpool.tile([O, 1], f32)
            nc.sync.dma_start(ob3c[:], ob3_d[:])

            # ACT / DVE balancer for PSUM-reading element ops
            busy = [0.0, 0.0]

            def pick(costs):
                e = min(range(2), key=lambda i: busy[i] + costs[i])
                busy[e] += costs[e]
                return e

            def relu_bias(dst, src, bias_col, cols):
                e = pick([cols / 1.2 + 180, cols / 0.96 + 125])
                if e == 0:
                    nc.scalar.activation(dst, src, AF.Relu, bias=bias_col)
                else:
                    nc.vector.tensor_scalar(dst, src, bias_col, 0.0, ALU.add, ALU.max)

            def copy_op(dst, src, cols):
                e = pick([cols / 1.2 + 180, cols / 0.96 + 125])
                if e == 0:
                    nc.scalar.activation(dst, src, AF.Copy)
                else:
                    nc.vector.tensor_copy(dst, src)

            import contextlib
            loop_cm = tc.For_i(0, reps, 1) if reps > 1 else contextlib.nullcontext()
            with loop_cm:
              augA = wpool.tile([128, bpc * N], bf16, tag="augA")
              aggTopA = wpool.tile([N, bpc * N], bf16, tag="aggTopA")
              for b in range(bpc):
                x_sb = wpool.tile([N, F], bf16, tag="x_sb")
                nc.sync.dma_start(x_sb[:], x_d[b])
                nc.sync.dma_start(augA[0:F, b * N:(b + 1) * N], xT_d[b])
                rt32 = wpool.tile([T * NST, 512], bf16, tag="rt32")
                nc.sync.dma_start(rt32[:], rt_d[b])

                aggT2 = wpool.tile([128, N], f32, tag="aggT2")
                for st in range(NST):
                    e0 = st * 512
                    # gather: pre^T = [senders^T ; receivers^T] for 512 pairs
                    prep = ppre.tile([128, 512], f32, tag="prep")
                    nc.tensor.matmul(
                        prep[0:64, :], x_sb[:], rsT[:, e0:e0 + 512],
                        start=True, stop=True,
                    )
                    nc.tensor.matmul(
                        prep[64:128, :], x_sb[:], rrT[:, e0:e0 + 512],
                        start=True, stop=True,
                    )
                    preT = wpool.tile([128, 512], bf16, tag="preT")
                    copy_op(preT[:], prep[:], 512)

                    for tp in range(2):
                        h1pair = []
                        for ti in range(2):
                            t = 2 * tp + ti
                            h1s = hpool.tile([128, 2 * 512], bf16, tag="h1s")
                            for hc in range(2):
                                h1p = ph1.tile([128, 512], f32, tag="h1p")
                                nc.tensor.matmul(
                                    h1p[:],
                                    w1s[:, t * H + hc * 128: t * H + (hc + 1) * 128],
                                    preT[:],
                                    start=True, stop=True,
                                )
                                relu_bias(
                                    h1s[:, hc * 512:(hc + 1) * 512], h1p[:],
                                    b1c[:, t * 2 + hc: t * 2 + hc + 1], 512,
                                )
                            h1pair.append(h1s)
                        # layer2 transposed, 2 types packed on partition halves
                        mTp = pmT.tile([128, 512], f32, tag="mTp")
                        for ti in range(2):
                            t = 2 * tp + ti
                            for kc in range(2):
                                nc.tensor.matmul(
                                    mTp[ti * 64:(ti + 1) * 64, :],
                                    w2s[:, (t * 2 + kc) * O:(t * 2 + kc + 1) * O],
                                    h1pair[ti][:, kc * 512:(kc + 1) * 512],
                                    start=(kc == 0), stop=(kc == 1),
                                    skip_group_check=True,
                                )
                        # rt rows for both types via one selector matmul
                        rtp = prt.tile([128, 512], f32, tag="rtp")
                        nc.tensor.matmul(
                            rtp[:],
                            sel2[:, (tp * NST + st) * 128:(tp * NST + st + 1) * 128],
                            rt32[:],
                            start=True, stop=True,
                        )
                        r2 = spool.tile([128, 512], bf16, tag="r2")
                        relu_bias(r2[:], mTp[:], b2p[:, tp:tp + 1], 512)
                        s = spool.tile([128, 512], bf16, tag="s")
                        nc.vector.tensor_tensor(s[:], r2[:], rtp[:], ALU.mult)
                        red = spool.tile([128, NST], f32, tag="red")
                        nc.vector.tensor_reduce(
                            red[:],
                            s[:].rearrange("p (j i) -> p j i", j=NST, i=N),
                            AX.X, ALU.add,
                        )
                        if tp == 0:
                            nc.gpsimd.tensor_copy(
                                aggT2[:, st * NST:(st + 1) * NST], red[:])
                        else:
                            nc.gpsimd.tensor_tensor(
                                aggT2[:, st * NST:(st + 1) * NST],
                                aggT2[:, st * NST:(st + 1) * NST], red[:],
                                ALU.add,
                            )

                # agg split: types 0+2 on partitions 0..63 -> aggTopA (base 0),
                # types 1+3 on partitions 64..127 -> augA agg half (lane-aligned)
                nc.gpsimd.tensor_copy(
                    aggTopA[:, b * N:(b + 1) * N], aggT2[0:N, :])
                nc.gpsimd.tensor_copy(
                    augA[F:128, b * N:(b + 1) * N], aggT2[N:128, :])

              # batched output MLP over all bpc batches (free = bpc*N = 512)
              W = bpc * N
              f1 = wpool.tile([128, 2 * W], bf16, tag="f1")
              for mc in range(2):
                  fp = pmT.tile([128, 512], f32, tag="mTp")
                  nc.tensor.matmul(
                      fp[:, 0:W], ow1s[:, mc * 128:(mc + 1) * 128], augA[:],
                      start=True, stop=False, skip_group_check=True,
                  )
                  nc.tensor.matmul(
                      fp[:, 0:W], ow1b0[:, mc * 128:(mc + 1) * 128], aggTopA[:],
                      start=False, stop=True, skip_group_check=True,
                  )
                  nc.scalar.activation(
                      f1[:, mc * W:(mc + 1) * W], fp[:, 0:W], AF.Relu,
                      bias=ob1c[:, mc:mc + 1],
                  )
              f2 = wpool.tile([128, 2 * W], bf16, tag="f2")
              for mc in range(2):
                  fp = pmT.tile([128, 512], f32, tag="mTp")
                  for kc in range(2):
                      nc.tensor.matmul(
                          fp[:, 0:W],
                          ow2s[:, kc * H + mc * 128: kc * H + (mc + 1) * 128],
                          f1[:, kc * W:(kc + 1) * W],
                          start=(kc == 0), stop=(kc == 1),
                      )
                  nc.scalar.activation(
                      f2[:, mc * W:(mc + 1) * W], fp[:, 0:W], AF.Relu,
                      bias=ob2c[:, mc:mc + 1],
                  )
              op = pmT.tile([128, 512], f32, tag="mTp")
              for kc in range(2):
                  nc.tensor.matmul(
                      op[0:O, 0:W], ow3s[:, kc * O:(kc + 1) * O],
                      f2[:, kc * W:(kc + 1) * W],
                      start=(kc == 0), stop=(kc == 1),
                  )
              yb = wpool.tile([O, W], f32, tag="yb")
              nc.vector.tensor_scalar(
                  yb[:], op[0:O, 0:W], ob3c[:, 0:1], None, ALU.add
              )
              nc.sync.dma_start(y_d[:], yb[:])

    nc.compile()
    return nc


def edge_maps(rel_rec, rel_send):
    """Pair-grid index (receiver-major) for each of the E directed edges."""
    send_idx = np.argmax(rel_send, axis=1).astype(np.int64)  # [E]
    rec_idx = np.argmax(rel_rec, axis=1).astype(np.int64)    # [E]
    return rec_idx * N + send_idx, rec_idx


def prep_shared(w1, b1, w2, b2, ow1, ob1, ow2, ob2, ow3, ob3):
    """Host-side layout prep for the replicated weights (bf16) + one-hots."""
    j_of = np.repeat(np.arange(N), N)         # receiver of pair e = j*64+i
    i_of = np.tile(np.arange(N), N)           # sender
    rsT = np.zeros((N, EP), np.float32)
    rsT[i_of, np.arange(EP)] = 1.0            # senders^T one-hot
    rrT = np.zeros((N, EP), np.float32)
    rrT[j_of, np.arange(EP)] = 1.0            # receivers^T one-hot
    R = T * NST
    sel2 = np.zeros((R, 2 * NST * 128), np.float32)
    for tp in range(2):
        for st in range(NST):
            base = (tp * NST + st) * 128
            sel2[2 * tp * NST + st, base:base + 64] = 1.0
            sel2[(2 * tp + 1) * NST + st, base + 64:base + 128] = 1.0
    w1s = np.ascontiguousarray(
        w1.transpose(1, 0, 2).reshape(2 * F, T * H)).astype(BF16)
    b1c = np.ascontiguousarray(
        b1.reshape(T, 2, 128).transpose(2, 0, 1).reshape(128, T * 2)
    ).astype(np.float32)
    w2s = np.ascontiguousarray(
        w2.reshape(T, 2, 128, O).transpose(2, 0, 1, 3).reshape(128, T * 2 * O)
    ).astype(BF16)
    b2pk = np.zeros((128, 2), np.float32)
    for tp in range(2):
        b2pk[0:64, tp] = b2[2 * tp]
        b2pk[64:128, tp] = b2[2 * tp + 1]
    ow1s = np.ascontiguousarray(ow1).astype(BF16)              # [128, H]
    ow1b0 = np.ascontiguousarray(ow1[N:2 * N]).astype(BF16)    # agg rows, base 0
    ob1c = np.ascontiguousarray(ob1.reshape(2, 128).T).astype(np.float32)
    ow2s = np.ascontiguousarray(
        ow2.reshape(2, 128, H).transpose(1, 0, 2).reshape(128, 2 * H)).astype(BF16)
    ob2c = np.ascontiguousarray(ob2.reshape(2, 128).T).astype(np.float32)
    ow3s = np.ascontiguousarray(
        ow3.reshape(2, 128, O).transpose(1, 0, 2).reshape(128, 2 * O)).astype(BF16)
    ob3c = np.ascontiguousarray(ob3.reshape(O, 1)).astype(np.float32)
    return dict(
        rsT=rsT.astype(BF16), rrT=rrT.astype(BF16), sel2=sel2.astype(BF16),
        w1s=w1s, b1c=b1c, w2s=w2s, b2p=b2pk,
        ow1s=ow1s, ow1b0=ow1b0, ob1c=ob1c, ow2s=ow2s, ob2c=ob2c,
        ow3s=ow3s, ob3c=ob3c,
    )


def prep_batch(x, rel_type, e_of_edge, rec_idx):
    """Per-core batched tensors: x, xT, rel_type as [T*NST, 512] rows."""
    bpc = x.shape[0]
    xT = np.ascontiguousarray(x.transpose(0, 2, 1))            # [bpc, F, N]
    rt_pad = np.zeros((bpc, EP, T), np.float32)
    rt_pad[:, e_of_edge, :] = rel_type                          # diag stays 0
    # row t*NST+st holds rel_type for pairs [st*512, (st+1)*512), type t
    rt32 = np.ascontiguousarray(
        rt_pad.reshape(bpc, NST, 512, T).transpose(0, 3, 1, 2).reshape(
            bpc, T * NST, 512)
    ).astype(BF16)
    return dict(x=np.ascontiguousarray(x).astype(BF16), xT=xT.astype(BF16),
                rt32=rt32)


def kernel(**inputs):
    from concourse.bass_utils import run_bass_kernel_spmd

    f32arrs = {k: np.asarray(v, dtype=np.float32) for k, v in inputs.items()}
    shared = prep_shared(
        f32arrs["w1"], f32arrs["b1"], f32arrs["w2"], f32arrs["b2"],
        f32arrs["ow1"], f32arrs["ob1"], f32arrs["ow2"], f32arrs["ob2"],
        f32arrs["ow3"], f32arrs["ob3"],
    )
    e_of_edge, rec_idx = edge_maps(f32arrs["rel_rec"], f32arrs["rel_send"])
    in_maps = []
    for c in range(NCORES):
        sl = slice(c * BPC, (c + 1) * BPC)
        m = dict(shared)
        m.update(prep_batch(
            f32arrs["x"][sl], f32arrs["rel_type"][sl], e_of_edge, rec_idx))
        in_maps.append(m)

    nc = build_nc(BPC)
    res = run_bass_kernel_spmd(nc, in_maps, list(range(NCORES)))
    # y per core: [O, BPC*N] -> [BPC, N, O]; concat -> full [B, N, O]
    y = np.concatenate(
        [res.results[c]["y"].reshape(O, BPC, N).transpose(1, 2, 0)
         for c in range(NCORES)], axis=0)
    return np.ascontiguousarray(y).astype(np.float32)


if __name__ == "__main__":
    # smoke: random inputs, shape check only
    rng = np.random.default_rng(0)
    eye = np.eye(N, dtype=np.float32)
    si, ri = [], []
    for i in range(N):
        for j in range(N):
            if i != j:
                si.append(i)
                ri.append(j)
    inputs = {
        "x": rng.standard_normal((B, N, F), dtype=np.float32),
        "rel_type": rng.random((B, E, T), dtype=np.float32),
        "rel_rec": eye[np.array(ri)],
        "rel_send": eye[np.array(si)],
        "w1": rng.standard_normal((T, 2 * F, H), dtype=np.float32) * 0.1,
        "b1": rng.standard_normal((T, H), dtype=np.float32) * 0.1,
        "w2": rng.standard_normal((T, H, O), dtype=np.float32) * 0.1,
        "b2": rng.standard_normal((T, O), dtype=np.float32) * 0.1,
        "ow1": rng.standard_normal((F + O, H), dtype=np.float32) * 0.1,
        "ob1": rng.standard_normal((H,), dtype=np.float32) * 0.1,
        "ow2": rng.standard_normal((H, H), dtype=np.float32) * 0.1,
        "ob2": rng.standard_normal((H,), dtype=np.float32) * 0.1,
        "ow3": rng.standard_normal((H, O), dtype=np.float32) * 0.1,
        "ob3": rng.standard_normal((O,), dtype=np.float32) * 0.1,
    }
    y = kernel(**inputs)
    print("y", y.shape, y.dtype)
